# revision 26
# baseline (speedup 1.0000x reference)
"""Expert-parallel Trainium2 Bass kernel for DeepEquiCategorySpecificMLP.

Routing (host): tokens sorted by cat_id; core c gets category c's tokens
(padded to PAD) + that category's weights, all bf16, feature-major
[feature, token].

Device pipeline (zero-bias fast path):
Every LayerNorm that precedes a matmul is folded INTO the matmul:
  LN(x) @ W  =  rstd ⊙ (x @ W  +  colsum(W) ⊗ (-mean))
The rank-1 centering term is appended to each PSUM accumulation group as a
K=1 matmul; the per-token rstd is applied lazily: relu(a*z) = a*relu(z) for
a>0 lets A1 ride through the relu, and LN scale-invariance makes the other
deferred scales cancel entirely.  The PE therefore streams all 224 main
matmuls back-to-back while stats (ones-vector matmuls, col-packed into
separate PE column strips) and row math (DVE, incl. bit-hack Newton rsqrt
to avoid scalar-engine activation-table swaps) run in parallel.
"""

import numpy as np
from contextlib import ExitStack

N_CORES = 8
D = 256
H = 1024
EPS = 1e-5
PAD_MIN = 288
KD, KH = D // 128, H // 128

_cache = {}


def _build(PAD, zbg, zbog, zb2):
    import concourse.bass as bass
    import concourse.tile as tile
    from concourse import bacc, mybir

    f32 = mybir.dt.float32
    f32r = mybir.dt.float32r
    bf = mybir.dt.bfloat16
    i32 = mybir.dt.int32
    AF = mybir.ActivationFunctionType
    ALU = mybir.AluOpType

    nc = bacc.Bacc("TRN2", target_bir_lowering=False, debug=False,
                   num_devices=N_CORES)

    xT_d = nc.dram_tensor("XT", [128, KD * PAD], bf, kind="ExternalInput")
    w0_d = nc.dram_tensor("W0", [128, KD * H], bf, kind="ExternalInput")
    wm_d = nc.dram_tensor("Wm", [128, KH * H], bf, kind="ExternalInput")
    f8 = mybir.dt.float8e4
    wg_d = nc.dram_tensor("Wg", [128, KH * H], f8, kind="ExternalInput")
    wog_d = nc.dram_tensor("Wog", [128, KH * H], bf, kind="ExternalInput")
    w2_d = nc.dram_tensor("W2", [128, KH * D], bf, kind="ExternalInput")
    rs_d = nc.dram_tensor("RS", [1, 2 * H + D], bf, kind="ExternalInput")
    need_bc = (not zbg) or (not zbog) or (not zb2)
    if need_bc:
        bc_d = nc.dram_tensor("BC", [128, 2 * KH + KD], f32,
                              kind="ExternalInput")
    out_d = nc.dram_tensor("out", [D, PAD], bf, kind="ExternalOutput")

    with ExitStack() as ctx:
        tc = ctx.enter_context(tile.TileContext(nc))
        wp = ctx.enter_context(tc.tile_pool(name="w", bufs=1))
        ap_ = ctx.enter_context(tc.tile_pool(name="a", bufs=1))
        rp = ctx.enter_context(tc.tile_pool(name="r", bufs=1))
        pmm = ctx.enter_context(
            tc.tile_pool(name="pmm", bufs=6, space=bass.MemorySpace.PSUM))
        pst = ctx.enter_context(
            tc.tile_pool(name="pst", bufs=2, space=bass.MemorySpace.PSUM))

        # ---------------- consts (warmup deps first) ----------------
        onesb = wp.tile([128, 1], bf, tag="onesb", name="onesb")
        nc.vector.memset(onesb[:], 1.0)
        warm = wp.tile([128, PAD], bf, tag="warm", name="warm")
        nc.vector.memset(warm[:], 0.0)
        onesf = wp.tile([128, 1], f32, tag="onesf", name="onesf")
        nc.vector.memset(onesf[:], 1.0)
        onesfr = wp.tile([128, 1], f32r, tag="onesfr", name="onesfr")
        nc.vector.tensor_copy(onesfr[:], onesf[:])
        onesr = wp.tile([1, 128], f32r, tag="onesr", name="onesr")
        nc.vector.tensor_copy(onesr[:], onesf[:1, :].broadcast_to([1, 128]))
        onesrb = wp.tile([1, 128], bf, tag="onesrb", name="onesrb")
        nc.vector.memset(onesrb[:], 1.0)
        crow = wp.tile([1, PAD], i32, tag="crow", name="crow")
        nc.vector.memset(crow[:], 0x5F3759DF)
        onei = wp.tile([1, PAD], i32, tag="onei", name="onei")
        nc.vector.memset(onei[:], 1)
        epsD = wp.tile([1, 1], f32, tag="epsD", name="epsD")
        nc.vector.memset(epsD[:], float(D) * float(D) * EPS)
        epsE = wp.tile([1, 1], f32, tag="epsE", name="epsE")
        nc.vector.memset(epsE[:], EPS)
        dum = wp.tile([1, 1], f32, tag="dum", name="dum")
        nc.vector.memset(dum[:], 0.0)

        # ---------------- input DMA ----------------
        # dram weights come host-interleaved as [128, K*F] so every
        # dma_start is a plain 2D copy (128 large contiguous descriptors)
        def load_flat(eng, dram, K, mfree, name, chunk, dt_=None):
            t = wp.tile([128, K * mfree], dt_ or bf, tag=name, name=name)
            for j in range(0, K, chunk):
                eng.dma_start(t[:, j * mfree:(j + chunk) * mfree],
                              dram.ap()[:, j * mfree:(j + chunk) * mfree])
            return [t[:, k * mfree:(k + 1) * mfree] for k in range(K)]

        # ONE queue, strict consumption order: a single HWDGE queue is
        # drained in order by the DMA engines, so arrival order is exactly
        # x, rs, w0, wg, wm, wog, w2 -- each layer's weights land just
        # before the PE needs them while earlier layers compute
        # x + W0 on the scalar HWDGE: generates in parallel with sync's
        # queue (which starts straight on wg), landing mm0's inputs ~1us
        # earlier; both queues start at the same post-barrier instant
        w0 = load_flat(nc.scalar, w0_d, KD, H, "w0", KD)
        xts = load_flat(nc.scalar, xT_d, KD, PAD, "xT", KD)
        rs = wp.tile([1, 2 * H + D], bf, tag="rs", name="rs")
        nc.sync.dma_start(rs[:], rs_d.ap())
        wgt = wp.tile([128, KH * H], f8, tag="wg", name="wg")
        nc.sync.dma_start(wgt[:, 0:4 * H], wg_d.ap()[:, 0:4 * H])
        nc.sync.dma_start(wgt[:, 4 * H:8 * H], wg_d.ap()[:, 4 * H:8 * H])
        wgr = wgt[:].rearrange("p (k m) -> p k m", k=KH)
        wm = load_flat(nc.sync, wm_d, KH, H, "wm", 4)
        wog = load_flat(nc.sync, wog_d, KH, H, "wog", 4)
        w2 = load_flat(nc.sync, w2_d, KH, D, "w2", KH)
        if need_bc:
            bct = wp.tile([128, 2 * KH + KD], f32, tag="bct", name="bct")
            nc.sync.dma_start(bct[:], bc_d.ap())
            bgc = bct[:, 0:KH]
            bogc = bct[:, KH:2 * KH]
            b2c = bct[:, 2 * KH:2 * KH + KD]

        # force the sigmoid act table as the initial load (first scalar act)
        dumo = rp.tile([1, 1], f32, tag="dumo", name="dumo")
        nc.scalar.activation(dumo[:], dum[:], AF.Sigmoid)

        # ---------------- PE warmup (HAM) ----------------
        warmS = pst.tile([64, PAD], f32, tag="st", name="warmS")
        for i in range(30):
            r = (i % 2) * 32
            nc.tensor.matmul(warmS[r:r + 1, :], onesb[:], warm[:],
                             start=True, stop=True)

        # ---------------- helpers ----------------
        def stats_pair(vals, sqs, name, ones=None):
            """Col-packed partition sums: row0 = colsum(vals),
            row32 = colsum(sqs).  vals/sqs: lists of [128, PAD] tiles."""
            if ones is None:
                ones = onesb
            S = pst.tile([64, PAD], f32, tag="st", name=name)
            K = len(vals)
            for k in range(K):
                nc.tensor.matmul(S[0:1, :], ones[:], vals[k],
                                 start=(k == 0), stop=(k == K - 1))
                if sqs is not None:
                    nc.tensor.matmul(S[32:33, :], ones[:], sqs[k],
                                     start=(k == 0), stop=(k == K - 1))
            return S

        def rsqrt_row(pref, u, iters, out_dt=f32):
            """y ~= u**-0.5 on DVE (quake seed + Newton), avoids scalar
            activation-table swaps.  u: [1, PAD] f32 SBUF tile AP."""
            ti = rp.tile([1, PAD], i32, tag=f"{pref}ti", name=f"{pref}ti")
            nc.vector.tensor_tensor(ti[:], u.bitcast(i32), onei[:],
                                    ALU.arith_shift_right)
            y = rp.tile([1, PAD], f32, tag=f"{pref}y0", name=f"{pref}y0")
            nc.vector.tensor_sub(y[:].bitcast(i32), crow[:], ti[:])
            cur = y
            for j in range(iters):
                a = rp.tile([1, PAD], f32, tag=f"{pref}a{j}",
                            name=f"{pref}a{j}")
                nc.vector.tensor_mul(a[:], cur[:], cur[:])
                nc.vector.tensor_mul(a[:], a[:], u)
                nc.vector.tensor_scalar(a[:], a[:], -0.5, 1.5,
                                        op0=ALU.mult, op1=ALU.add)
                y2 = rp.tile([1, PAD], out_dt if j == iters - 1 else f32,
                             tag=f"{pref}y{j+1}", name=f"{pref}y{j+1}")
                nc.vector.tensor_mul(y2[:], a[:], cur[:])
                cur = y2
            return cur

        def mm_layer(wtiles, atiles, MT, mgroup, rank1, evac):
            """Main matmul layer with optional per-m rank-1 correction
            appended to the accumulation group.  rank1 = (stat_row_fn, mrow)
            where stat_row_fn(m) gives the [1,128] stationary slice."""
            outs = []
            K = len(atiles)
            for g0 in range(0, MT, mgroup):
                ms = list(range(g0, min(g0 + mgroup, MT)))
                pss = [pmm.tile([128, PAD], f32, tag="mm", name=f"mm{m}")
                       for m in ms]
                last = (rank1 is None)
                for k in range(K):
                    for i, m in enumerate(ms):
                        nc.tensor.matmul(
                            pss[i][:],
                            wtiles[k][:, m * 128:(m + 1) * 128],
                            atiles[k],
                            start=(k == 0), stop=(last and k == K - 1))
                if rank1 is not None:
                    statf, mrow = rank1
                    for i, m in enumerate(ms):
                        nc.tensor.matmul(pss[i][:], statf(m), mrow[:],
                                         start=False, stop=True)
                for i, m in enumerate(ms):
                    outs.append(evac(m, pss[i]))
            return outs

        # ---------------- input LN stats (on raw bf16 x) ----------------
        sqx = []
        for k in range(KD):
            t = ap_.tile([128, PAD], bf, tag=f"sqx{k}", name=f"sqx{k}")
            nc.vector.tensor_mul(t[:], xts[k], xts[k])
            sqx.append(t[:])
        Sx = stats_pair(xts, sqx, "Sx")
        # r1row = -mean1 (bf16, moving row of the mm0 rank-1)
        r1row = rp.tile([1, PAD], bf, tag="r1row", name="r1row")
        nc.vector.tensor_scalar(r1row[:], Sx[0:1, :], -1.0 / D, None,
                                op0=ALU.mult)
        t1 = rp.tile([1, PAD], f32, tag="t1x", name="t1x")
        nc.vector.tensor_mul(t1[:], r1row[:], r1row[:])
        u1 = rp.tile([1, PAD], f32, tag="u1", name="u1")
        nc.vector.scalar_tensor_tensor(u1[:], Sx[32:33, :], 1.0 / D, t1[:],
                                       op0=ALU.mult, op1=ALU.subtract)
        nc.vector.tensor_scalar(u1[:], u1[:], 64.0, 64.0 * EPS,
                                op0=ALU.mult, op1=ALU.add)
        A1 = rsqrt_row("A1", u1[:], iters=1)
        A1b = ap_.tile([128, PAD], f32, tag="A1b", name="A1b")
        nc.gpsimd.partition_broadcast(A1b[:], A1[:])

        # ---------------- mm0: t0 = relu(W0^T x + wsum0 (x) r1row) --------
        t0f8 = ap_.tile([128, KH * PAD], f8, tag="t0f8", name="t0f8")
        t0f8r = t0f8[:].rearrange("p (k n) -> p k n", k=KH)

        def evac_relu(m, ps):
            t = ap_.tile([128, PAD], bf, tag=f"t0{m}", name=f"t0{m}")
            nc.scalar.activation(t[:], ps[:], AF.Relu)
            nc.vector.tensor_copy(t0f8[:, m * PAD:(m + 1) * PAD], t[:])
            return t[:]

        t0 = mm_layer(w0, xts, KH, 4,
                      (lambda m: rs[:, m * 128:(m + 1) * 128], r1row),
                      evac_relu)

        # ---------------- gate: gsig = sigmoid(A1 * (Wg^T t0)) ------------
        def evac_gate(m, ps):
            tmp = ap_.tile([128, PAD], bf, tag=f"gt{m}", name=f"gt{m}")
            nc.vector.tensor_mul(tmp[:], ps[:], A1b[:])
            if not zbg:
                nc.vector.tensor_scalar(tmp[:], tmp[:], bgc[:, m:m + 1],
                                        None, op0=ALU.add)
            g = ap_.tile([128, PAD], bf, tag=f"gs{m}", name=f"gs{m}")
            nc.scalar.activation(g[:], tmp[:], AF.Sigmoid)
            return g[:]

        DR = mybir.MatmulPerfMode.DoubleRow
        gsig = []
        for g0 in (0, 4):
            ms = list(range(g0, g0 + 4))
            pss = [pmm.tile([128, PAD], f32, tag="mm", name=f"g{m}")
                   for m in ms]
            for kp in range(0, KH, 2):
                for i, m in enumerate(ms):
                    nc.tensor.matmul(pss[i][:],
                                     wgr[:, kp:kp + 2, m * 128:(m + 1) * 128],
                                     t0f8r[:, kp:kp + 2, :],
                                     start=(kp == 0), stop=(kp == KH - 2),
                                     perf_mode=DR)
            for i, m in enumerate(ms):
                gsig.append(evac_gate(m, pss[i]))

        # ---------------- main: gated_t = (Wm^T t0) * gsig ----------------
        def evac_main(m, ps):
            t = ap_.tile([128, PAD], bf, tag=f"gd{m}", name=f"gd{m}")
            nc.vector.tensor_mul(t[:], ps[:], gsig[m])
            return t[:]

        gated = mm_layer(wm, t0, KH, 4, None, evac_main)

        # ---------------- og + gated-LN (center only; scales wash out) ----
        # og k-MMs are emitted before the gated mean-stats so the PE is not
        # idle while the gated tail evacuates; only the rank-1 close needs
        # the stats row.
        t3, sq3 = [], []

        def evac_og(m, ps):
            t = ap_.tile([128, PAD], bf, tag=f"t3{m}", name=f"t3{m}")
            if zbog:
                nc.scalar.activation(t[:], ps[:], AF.Identity)
            else:
                nc.scalar.activation(t[:], ps[:], AF.Identity,
                                     bias=bogc[:, m:m + 1])
            s = ap_.tile([128, PAD], bf, tag=f"sq3{m}", name=f"sq3{m}")
            nc.vector.tensor_mul(s[:], t[:], t[:])
            t3.append(t[:])
            sq3.append(s[:])

        rgrow = rp.tile([1, PAD], bf, tag="rgrow", name="rgrow")
        for g0 in (0, 4):
            ms = list(range(g0, g0 + 4))
            pss = [pmm.tile([128, PAD], f32, tag="mm", name=f"og{m}")
                   for m in ms]
            for k in range(KH):
                for i, m in enumerate(ms):
                    nc.tensor.matmul(pss[i][:],
                                     wog[k][:, m * 128:(m + 1) * 128],
                                     gated[k], start=(k == 0), stop=False)
            if g0 == 0:
                Sg = pst.tile([64, PAD], f32, tag="st", name="Sg")
                for k in range(4):
                    nc.tensor.matmul(Sg[0:1, :], onesb[:], gated[k],
                                     start=(k == 0), stop=(k == 3))
                for k in range(4, 8):
                    nc.tensor.matmul(Sg[32:33, :], onesb[:], gated[k],
                                     start=(k == 4), stop=(k == 7))
                cg = rp.tile([1, PAD], f32, tag="cg", name="cg")
                nc.vector.tensor_scalar(cg[:], Sg[32:33, :], -1.0 / H,
                                        None, op0=ALU.mult)
                nc.vector.scalar_tensor_tensor(rgrow[:], Sg[0:1, :],
                                               -1.0 / H, cg[:],
                                               op0=ALU.mult, op1=ALU.add)
            for i, m in enumerate(ms):
                nc.tensor.matmul(pss[i][:],
                                 rs[:, H + m * 128:H + (m + 1) * 128],
                                 rgrow[:], start=False, stop=True)
            for i, m in enumerate(ms):
                evac_og(m, pss[i])

        # hoist the sigmoid->abs_rsqrt activation-table swap here (hidden
        # under og/mm2 PE work); without this the auto-inserted load lands
        # after t13, which depends on the late h2 stats
        dumo2 = rp.tile([1, 1], f32, tag="dumo2", name="dumo2")
        nc.scalar.activation(dumo2[:], dum[:], AF.Abs_reciprocal_sqrt,
                             bias=epsE[:])

        # ---------------- mm2 k-MMs interleaved with the h2 stats ---------
        # first half covers the og-evac tail, stats run next, second half
        # covers the A3 row chain; only the rank-1 close needs the rows
        ps2 = [pmm.tile([128, PAD], f32, tag="mm", name=f"y{m}")
               for m in range(KD)]
        for k in range(KH // 2):
            for m in range(KD):
                nc.tensor.matmul(ps2[m][:],
                                 w2[k][:, m * 128:(m + 1) * 128], t3[k],
                                 start=(k == 0), stop=False)

        S3 = stats_pair(t3, sq3, "S3")

        for k in range(KH // 2, KH):
            for m in range(KD):
                nc.tensor.matmul(ps2[m][:],
                                 w2[k][:, m * 128:(m + 1) * 128], t3[k],
                                 start=False, stop=False)
        r2row = rp.tile([1, PAD], bf, tag="r2row", name="r2row")
        nc.vector.tensor_scalar(r2row[:], S3[0:1, :], -1.0 / H, None,
                                op0=ALU.mult)
        t13 = rp.tile([1, PAD], f32, tag="t13", name="t13")
        nc.scalar.activation(t13[:], S3[0:1, :], AF.Square, scale=1.0 / H)
        u3 = rp.tile([1, PAD], f32, tag="u3", name="u3")
        nc.vector.scalar_tensor_tensor(u3[:], S3[32:33, :], 1.0 / H, t13[:],
                                       op0=ALU.mult, op1=ALU.subtract)
        # scalar-engine rsqrt: the act-table swap (sigmoid -> abs_rsqrt,
        # auto-inserted after the last t3 copy) hides under mm_og/mm2
        A3 = rp.tile([1, PAD], f32, tag="A3", name="A3")
        nc.scalar.activation(A3[:], u3[:], AF.Abs_reciprocal_sqrt,
                             bias=epsE[:])
        A3b = ap_.tile([128, PAD], f32, tag="A3b", name="A3b")
        nc.gpsimd.partition_broadcast(A3b[:], A3[:])

        # ---------------- mm2 rank-1 close + output path ------------------
        opre, sq4 = [], []

        def evac_y(m, ps):
            o = ap_.tile([128, PAD], f32r, tag=f"o{m}", name=f"o{m}")
            nc.vector.tensor_mul(o[:], ps[:], A3b[:])
            if not zb2:
                nc.vector.tensor_scalar(o[:], o[:], b2c[:, m:m + 1], None,
                                        op0=ALU.add)
            op = ap_.tile([128, PAD], f32r, tag=f"op{m}", name=f"op{m}")
            nc.vector.scalar_tensor_tensor(op[:], xts[m], 0.1, o[:],
                                           op0=ALU.mult, op1=ALU.add)
            s4 = ap_.tile([128, PAD], f32r, tag=f"s4{m}", name=f"s4{m}")
            nc.scalar.activation(s4[:], op[:], AF.Square)
            opre.append(op[:])
            sq4.append(s4[:])

        for m in range(KD):
            nc.tensor.matmul(ps2[m][:],
                             rs[:, 2 * H + m * 128:2 * H + (m + 1) * 128],
                             r2row[:], start=False, stop=True)
        for m in range(KD):
            evac_y(m, ps2[m])

        # ---------------- final LN (exact, scalar rsqrt) ------------------
        S4a = pst.tile([64, PAD], f32, tag="st", name="S4a")
        S4b = pst.tile([64, PAD], f32, tag="st", name="S4b")
        for k in range(KD):
            nc.tensor.matmul(S4a[0:1, :], onesfr[:], opre[k],
                             start=(k == 0), stop=(k == KD - 1))
            nc.tensor.matmul(S4b[0:1, :], onesfr[:], sq4[k],
                             start=(k == 0), stop=(k == KD - 1))
        # out = rr4 * (D*opre - s1): broadcast s1 and rr4 separately so the
        # s1 broadcast overlaps the u4/rr4 row chain
        s1s = rp.tile([1, PAD], bf, tag="s1s", name="s1s")
        nc.vector.tensor_copy(s1s[:], S4a[0:1, :])
        s1b = pmm.tile([128, PAD], f32, tag="mm", name="s1b")
        nc.tensor.matmul(s1b[:], onesrb[:], s1s[:], start=True, stop=True)
        t14 = rp.tile([1, PAD], f32, tag="t14", name="t14")
        nc.scalar.activation(t14[:], S4a[0:1, :], AF.Square)
        u4 = rp.tile([1, PAD], f32, tag="u4", name="u4")
        nc.vector.scalar_tensor_tensor(u4[:], S4b[0:1, :], float(D), t14[:],
                                       op0=ALU.mult, op1=ALU.subtract)
        rr4 = rp.tile([1, PAD], bf, tag="rr4", name="rr4")
        nc.scalar.activation(rr4[:], u4[:], AF.Abs_reciprocal_sqrt,
                             bias=epsD[:])
        r4b = pmm.tile([128, PAD], f32, tag="mm", name="r4b")
        nc.tensor.matmul(r4b[:], onesrb[:], rr4[:], start=True, stop=True)
        for k in range(KD):
            ot = ap_.tile([128, PAD], bf, tag=f"ot{k}", name=f"ot{k}")
            nc.vector.scalar_tensor_tensor(ot[:], opre[k], float(D),
                                           s1b[:], op0=ALU.mult,
                                           op1=ALU.subtract)
            nc.vector.tensor_mul(ot[:], ot[:], r4b[:])
            (nc.sync if k == 0 else nc.scalar).dma_start(
                out_d.ap()[k * 128:(k + 1) * 128, :], ot[:])

    nc.compile()
    return nc


def _get_nc(PAD, zbg, zbog, zb2):
    key = (PAD, zbg, zbog, zb2)
    if key not in _cache:
        _cache[key] = _build(PAD, zbg, zbog, zb2)
    return _cache[key]


def _bf16(a):
    import ml_dtypes
    return np.ascontiguousarray(
        np.asarray(a, dtype=np.float32).astype(ml_dtypes.bfloat16))


def _f8(a, scale):
    import ml_dtypes
    return np.ascontiguousarray(
        (np.asarray(a, dtype=np.float32) * scale).astype(
            ml_dtypes.float8_e4m3))


def _flatk(w):
    """[K*128, F] -> [128, K*F]: per-partition concat of the K k-tiles so
    the device DMA is one big contiguous descriptor per partition."""
    K = w.shape[0] // 128
    return np.ascontiguousarray(
        w.reshape(K, 128, -1).transpose(1, 0, 2).reshape(128, -1))


def _numpy_ref(x, cat_ids, W0, b0, Wm, bm, Wg, bg, Wog, bog, W2, b2):
    """Host fallback for the (never-hit) nonzero b0/bm case."""
    def ln(v):
        m = v.mean(-1, keepdims=True)
        s = ((v - m) ** 2).mean(-1, keepdims=True)
        return (v - m) / np.sqrt(s + EPS)

    x = np.asarray(x, np.float32)
    cid = np.asarray(cat_ids).astype(np.int64).ravel()
    xn = ln(x)
    out = np.zeros_like(x)
    for c in range(N_CORES):
        idx = np.where(cid == c)[0]
        if len(idx) == 0:
            continue
        h = np.maximum(xn[idx] @ W0[c] + b0[c], 0)
        main = h @ Wm[c] + bm[c]
        gate = h @ Wg[c] + bg[c]
        g = ln(main * (1.0 / (1.0 + np.exp(-gate))))
        h = ln(g @ Wog[c] + bog[c])
        out[idx] = h @ W2[c] + b2[c]
    return ln(out + 0.1 * x).astype(np.float32)


def _prep(x, cat_ids, W0, b0, Wm, bm, Wg, bg, Wog, bog, W2, b2):
    x = np.ascontiguousarray(np.asarray(x, dtype=np.float32))
    cid = np.asarray(cat_ids).astype(np.int64).ravel()
    counts = np.bincount(cid, minlength=N_CORES)
    PAD = int(max(PAD_MIN, ((counts.max() + 31) // 32) * 32))
    order = np.argsort(cid, kind="stable")
    starts = np.zeros(N_CORES + 1, np.int64)
    starts[1:] = np.cumsum(counts)

    zbg = not np.any(np.asarray(bg))
    zbog = not np.any(np.asarray(bog))
    zb2 = not np.any(np.asarray(b2))
    need_bc = (not zbg) or (not zbog) or (not zb2)

    in_maps = []
    for c in range(N_CORES):
        ids = order[starts[c]:starts[c + 1]]
        xc = np.zeros((PAD, D), np.float32)
        xc[:len(ids)] = x[ids]
        w0b = _bf16(W0[c])
        wogb = _bf16(Wog[c])
        w2b = _bf16(W2[c])
        rsum = np.concatenate([
            w0b.astype(np.float32).sum(0),
            wogb.astype(np.float32).sum(0),
            w2b.astype(np.float32).sum(0),
        ])[None, :]
        m = {
            "XT": _flatk(_bf16(xc.T)),
            "W0": _flatk(w0b),
            "Wm": _flatk(_bf16(Wm[c])),
            "Wg": _flatk(_f8(Wg[c], 8.0)),
            "Wog": _flatk(wogb), "W2": _flatk(w2b),
            "RS": _bf16(rsum),
        }
        if need_bc:
            bc = np.concatenate([
                np.asarray(bg[c], np.float32).reshape(KH, 128).T,
                np.asarray(bog[c], np.float32).reshape(KH, 128).T,
                np.asarray(b2[c], np.float32).reshape(KD, 128).T,
            ], axis=1)
            m["BC"] = np.ascontiguousarray(bc)
        in_maps.append(m)
    return in_maps, order, starts, PAD, (zbg, zbog, zb2), x.shape[0]


def kernel(x, cat_ids, W0, b0, Wm, bm, Wg, bg, Wog, bog, W2, b2,
           **run_kwargs):
    if np.any(np.asarray(b0)) or np.any(np.asarray(bm)):
        return _numpy_ref(x, cat_ids, W0, b0, Wm, bm, Wg, bg, Wog, bog,
                          W2, b2)
    from concourse.bass_utils import run_bass_kernel_spmd

    in_maps, order, starts, PAD, flags, N = _prep(
        x, cat_ids, W0, b0, Wm, bm, Wg, bg, Wog, bog, W2, b2)
    nc = _get_nc(PAD, *flags)
    res = run_bass_kernel_spmd(nc, in_maps, core_ids=list(range(N_CORES)),
                               **run_kwargs)
    out = np.zeros((N, D), np.float32)
    for c in range(N_CORES):
        ids = order[starts[c]:starts[c + 1]]
        out[ids] = res.results[c]["out"].astype(np.float32).T[:len(ids)]
    if run_kwargs:
        kernel.last_results = res
    return out


# revision 28
# speedup vs baseline: 1.0087x; 1.0087x over previous
"""Expert-parallel Trainium2 Bass kernel for DeepEquiCategorySpecificMLP.

Routing (host): tokens sorted by cat_id; core c gets category c's tokens
(padded to PAD) + that category's weights, all bf16, feature-major
[feature, token].

Device pipeline (zero-bias fast path):
Every LayerNorm that precedes a matmul is folded INTO the matmul:
  LN(x) @ W  =  rstd ⊙ (x @ W  +  colsum(W) ⊗ (-mean))
The rank-1 centering term is appended to each PSUM accumulation group as a
K=1 matmul; the per-token rstd is applied lazily: relu(a*z) = a*relu(z) for
a>0 lets A1 ride through the relu, and LN scale-invariance makes the other
deferred scales cancel entirely.  The PE therefore streams all 224 main
matmuls back-to-back while stats (ones-vector matmuls, col-packed into
separate PE column strips) and row math (DVE, incl. bit-hack Newton rsqrt
to avoid scalar-engine activation-table swaps) run in parallel.
"""

import numpy as np
from contextlib import ExitStack

N_CORES = 8
D = 256
H = 1024
EPS = 1e-5
PAD_MIN = 288
KD, KH = D // 128, H // 128

_cache = {}


def _build(PAD, zbg, zbog, zb2):
    import concourse.bass as bass
    import concourse.tile as tile
    from concourse import bacc, mybir

    f32 = mybir.dt.float32
    f32r = mybir.dt.float32r
    bf = mybir.dt.bfloat16
    i32 = mybir.dt.int32
    AF = mybir.ActivationFunctionType
    ALU = mybir.AluOpType

    nc = bacc.Bacc("TRN2", target_bir_lowering=False, debug=False,
                   num_devices=N_CORES)

    xT_d = nc.dram_tensor("XT", [128, KD * PAD], bf, kind="ExternalInput")
    w0_d = nc.dram_tensor("W0", [128, KD * H], bf, kind="ExternalInput")
    wm_d = nc.dram_tensor("Wm", [128, KH * H], bf, kind="ExternalInput")
    f8 = mybir.dt.float8e4
    wg_d = nc.dram_tensor("Wg", [128, KH * H], f8, kind="ExternalInput")
    wog_d = nc.dram_tensor("Wog", [128, KH * H], bf, kind="ExternalInput")
    w2_d = nc.dram_tensor("W2", [128, KH * D], bf, kind="ExternalInput")
    rs_d = nc.dram_tensor("RS", [1, 2 * H + D], bf, kind="ExternalInput")
    need_bc = (not zbg) or (not zbog) or (not zb2)
    if need_bc:
        bc_d = nc.dram_tensor("BC", [128, 2 * KH + KD], f32,
                              kind="ExternalInput")
    out_d = nc.dram_tensor("out", [D, PAD], bf, kind="ExternalOutput")

    with ExitStack() as ctx:
        tc = ctx.enter_context(tile.TileContext(nc))
        wp = ctx.enter_context(tc.tile_pool(name="w", bufs=1))
        ap_ = ctx.enter_context(tc.tile_pool(name="a", bufs=1))
        rp = ctx.enter_context(tc.tile_pool(name="r", bufs=1))
        pmm = ctx.enter_context(
            tc.tile_pool(name="pmm", bufs=6, space=bass.MemorySpace.PSUM))
        pst = ctx.enter_context(
            tc.tile_pool(name="pst", bufs=2, space=bass.MemorySpace.PSUM))

        # ---------------- consts (warmup deps first) ----------------
        onesb = wp.tile([128, 1], bf, tag="onesb", name="onesb")
        nc.vector.memset(onesb[:], 1.0)
        warm = wp.tile([128, PAD], bf, tag="warm", name="warm")
        nc.vector.memset(warm[:], 0.0)
        onesf = wp.tile([128, 1], f32, tag="onesf", name="onesf")
        nc.vector.memset(onesf[:], 1.0)
        onesfr = wp.tile([128, 1], f32r, tag="onesfr", name="onesfr")
        nc.vector.tensor_copy(onesfr[:], onesf[:])
        onesr = wp.tile([1, 128], f32r, tag="onesr", name="onesr")
        nc.vector.tensor_copy(onesr[:], onesf[:1, :].broadcast_to([1, 128]))
        onesrb = wp.tile([1, 128], bf, tag="onesrb", name="onesrb")
        nc.vector.memset(onesrb[:], 1.0)
        crow = wp.tile([1, PAD], i32, tag="crow", name="crow")
        nc.vector.memset(crow[:], 0x5F3759DF)
        onei = wp.tile([1, PAD], i32, tag="onei", name="onei")
        nc.vector.memset(onei[:], 1)
        epsD = wp.tile([1, 1], f32, tag="epsD", name="epsD")
        nc.vector.memset(epsD[:], float(D) * float(D) * EPS)
        epsE = wp.tile([1, 1], f32, tag="epsE", name="epsE")
        nc.vector.memset(epsE[:], EPS)
        dum = wp.tile([1, 1], f32, tag="dum", name="dum")
        nc.vector.memset(dum[:], 0.0)

        # ---------------- input DMA ----------------
        # dram weights come host-interleaved as [128, K*F] so every
        # dma_start is a plain 2D copy (128 large contiguous descriptors)
        def load_flat(eng, dram, K, mfree, name, chunk, dt_=None):
            t = wp.tile([128, K * mfree], dt_ or bf, tag=name, name=name)
            for j in range(0, K, chunk):
                eng.dma_start(t[:, j * mfree:(j + chunk) * mfree],
                              dram.ap()[:, j * mfree:(j + chunk) * mfree])
            return [t[:, k * mfree:(k + 1) * mfree] for k in range(K)]

        # ONE queue, strict consumption order: a single HWDGE queue is
        # drained in order by the DMA engines, so arrival order is exactly
        # x, rs, w0, wg, wm, wog, w2 -- each layer's weights land just
        # before the PE needs them while earlier layers compute
        w0 = load_flat(nc.sync, w0_d, KD, H, "w0", KD)
        xts = load_flat(nc.sync, xT_d, KD, PAD, "xT", KD)
        rs = wp.tile([1, 2 * H + D], bf, tag="rs", name="rs")
        nc.sync.dma_start(rs[:], rs_d.ap())
        wgt = wp.tile([128, KH * H], f8, tag="wg", name="wg")
        nc.sync.dma_start(wgt[:, 0:4 * H], wg_d.ap()[:, 0:4 * H])
        nc.sync.dma_start(wgt[:, 4 * H:8 * H], wg_d.ap()[:, 4 * H:8 * H])
        wgr = wgt[:].rearrange("p (k m) -> p k m", k=KH)
        wm = load_flat(nc.sync, wm_d, KH, H, "wm", 4)
        wog = load_flat(nc.sync, wog_d, KH, H, "wog", 4)
        w2 = load_flat(nc.sync, w2_d, KH, D, "w2", KH)
        if need_bc:
            bct = wp.tile([128, 2 * KH + KD], f32, tag="bct", name="bct")
            nc.sync.dma_start(bct[:], bc_d.ap())
            bgc = bct[:, 0:KH]
            bogc = bct[:, KH:2 * KH]
            b2c = bct[:, 2 * KH:2 * KH + KD]

        # force the sigmoid act table as the initial load (first scalar act)
        dumo = rp.tile([1, 1], f32, tag="dumo", name="dumo")
        nc.scalar.activation(dumo[:], dum[:], AF.Sigmoid)

        # ---------------- PE warmup (HAM) ----------------
        warmS = pst.tile([64, PAD], f32, tag="st", name="warmS")
        for i in range(30):
            r = (i % 2) * 32
            nc.tensor.matmul(warmS[r:r + 1, :], onesb[:], warm[:],
                             start=True, stop=True)

        # ---------------- helpers ----------------
        def stats_pair(vals, sqs, name, ones=None):
            """Col-packed partition sums: row0 = colsum(vals),
            row32 = colsum(sqs).  vals/sqs: lists of [128, PAD] tiles."""
            if ones is None:
                ones = onesb
            S = pst.tile([64, PAD], f32, tag="st", name=name)
            K = len(vals)
            for k in range(K):
                nc.tensor.matmul(S[0:1, :], ones[:], vals[k],
                                 start=(k == 0), stop=(k == K - 1))
                if sqs is not None:
                    nc.tensor.matmul(S[32:33, :], ones[:], sqs[k],
                                     start=(k == 0), stop=(k == K - 1))
            return S

        def rsqrt_row(pref, u, iters, out_dt=f32):
            """y ~= u**-0.5 on DVE (quake seed + Newton), avoids scalar
            activation-table swaps.  u: [1, PAD] f32 SBUF tile AP."""
            ti = rp.tile([1, PAD], i32, tag=f"{pref}ti", name=f"{pref}ti")
            nc.vector.tensor_tensor(ti[:], u.bitcast(i32), onei[:],
                                    ALU.arith_shift_right)
            y = rp.tile([1, PAD], f32, tag=f"{pref}y0", name=f"{pref}y0")
            nc.vector.tensor_sub(y[:].bitcast(i32), crow[:], ti[:])
            cur = y
            for j in range(iters):
                a = rp.tile([1, PAD], f32, tag=f"{pref}a{j}",
                            name=f"{pref}a{j}")
                nc.vector.tensor_mul(a[:], cur[:], cur[:])
                nc.vector.tensor_mul(a[:], a[:], u)
                nc.vector.tensor_scalar(a[:], a[:], -0.5, 1.5,
                                        op0=ALU.mult, op1=ALU.add)
                y2 = rp.tile([1, PAD], out_dt if j == iters - 1 else f32,
                             tag=f"{pref}y{j+1}", name=f"{pref}y{j+1}")
                nc.vector.tensor_mul(y2[:], a[:], cur[:])
                cur = y2
            return cur

        def mm_layer(wtiles, atiles, MT, mgroup, rank1, evac):
            """Main matmul layer with optional per-m rank-1 correction
            appended to the accumulation group.  rank1 = (stat_row_fn, mrow)
            where stat_row_fn(m) gives the [1,128] stationary slice."""
            outs = []
            K = len(atiles)
            for g0 in range(0, MT, mgroup):
                ms = list(range(g0, min(g0 + mgroup, MT)))
                pss = [pmm.tile([128, PAD], f32, tag="mm", name=f"mm{m}")
                       for m in ms]
                last = (rank1 is None)
                for k in range(K):
                    for i, m in enumerate(ms):
                        nc.tensor.matmul(
                            pss[i][:],
                            wtiles[k][:, m * 128:(m + 1) * 128],
                            atiles[k],
                            start=(k == 0), stop=(last and k == K - 1))
                if rank1 is not None:
                    statf, mrow = rank1
                    for i, m in enumerate(ms):
                        nc.tensor.matmul(pss[i][:], statf(m), mrow[:],
                                         start=False, stop=True)
                for i, m in enumerate(ms):
                    outs.append(evac(m, pss[i]))
            return outs

        # ---------------- input LN stats (on raw bf16 x) ----------------
        sqx = []
        for k in range(KD):
            t = ap_.tile([128, PAD], bf, tag=f"sqx{k}", name=f"sqx{k}")
            nc.vector.tensor_mul(t[:], xts[k], xts[k])
            sqx.append(t[:])
        Sx = stats_pair(xts, sqx, "Sx")
        # r1row = -mean1 (bf16, moving row of the mm0 rank-1)
        r1row = rp.tile([1, PAD], bf, tag="r1row", name="r1row")
        nc.vector.tensor_scalar(r1row[:], Sx[0:1, :], -1.0 / D, None,
                                op0=ALU.mult)
        t1 = rp.tile([1, PAD], f32, tag="t1x", name="t1x")
        nc.vector.tensor_mul(t1[:], r1row[:], r1row[:])
        u1 = rp.tile([1, PAD], f32, tag="u1", name="u1")
        nc.vector.scalar_tensor_tensor(u1[:], Sx[32:33, :], 1.0 / D, t1[:],
                                       op0=ALU.mult, op1=ALU.subtract)
        nc.vector.tensor_scalar(u1[:], u1[:], 64.0, 64.0 * EPS,
                                op0=ALU.mult, op1=ALU.add)
        A1 = rsqrt_row("A1", u1[:], iters=1)
        A1b = ap_.tile([128, PAD], f32, tag="A1b", name="A1b")
        nc.gpsimd.partition_broadcast(A1b[:], A1[:])

        # ---------------- mm0: t0 = relu(W0^T x + wsum0 (x) r1row) --------
        t0f8 = ap_.tile([128, KH * PAD], f8, tag="t0f8", name="t0f8")
        t0f8r = t0f8[:].rearrange("p (k n) -> p k n", k=KH)

        def evac_relu(m, ps):
            t = ap_.tile([128, PAD], bf, tag=f"t0{m}", name=f"t0{m}")
            nc.scalar.activation(t[:], ps[:], AF.Relu)
            nc.vector.tensor_copy(t0f8[:, m * PAD:(m + 1) * PAD], t[:])
            return t[:]

        t0 = mm_layer(w0, xts, KH, 4,
                      (lambda m: rs[:, m * 128:(m + 1) * 128], r1row),
                      evac_relu)

        # ---------------- gate: gsig = sigmoid(A1 * (Wg^T t0)) ------------
        def evac_gate(m, ps):
            tmp = ap_.tile([128, PAD], bf, tag=f"gt{m}", name=f"gt{m}")
            nc.vector.tensor_mul(tmp[:], ps[:], A1b[:])
            if not zbg:
                nc.vector.tensor_scalar(tmp[:], tmp[:], bgc[:, m:m + 1],
                                        None, op0=ALU.add)
            g = ap_.tile([128, PAD], bf, tag=f"gs{m}", name=f"gs{m}")
            nc.scalar.activation(g[:], tmp[:], AF.Sigmoid)
            return g[:]

        DR = mybir.MatmulPerfMode.DoubleRow
        gsig = []
        for g0 in (0, 4):
            ms = list(range(g0, g0 + 4))
            pss = [pmm.tile([128, PAD], f32, tag="mm", name=f"g{m}")
                   for m in ms]
            for kp in range(0, KH, 2):
                for i, m in enumerate(ms):
                    nc.tensor.matmul(pss[i][:],
                                     wgr[:, kp:kp + 2, m * 128:(m + 1) * 128],
                                     t0f8r[:, kp:kp + 2, :],
                                     start=(kp == 0), stop=(kp == KH - 2),
                                     perf_mode=DR)
            for i, m in enumerate(ms):
                gsig.append(evac_gate(m, pss[i]))

        # ---------------- main: gated_t = (Wm^T t0) * gsig ----------------
        def evac_main(m, ps):
            t = ap_.tile([128, PAD], bf, tag=f"gd{m}", name=f"gd{m}")
            nc.vector.tensor_mul(t[:], ps[:], gsig[m])
            return t[:]

        gated = mm_layer(wm, t0, KH, 4, None, evac_main)

        # ---------------- og + gated-LN (center only; scales wash out) ----
        # og k-MMs are emitted before the gated mean-stats so the PE is not
        # idle while the gated tail evacuates; only the rank-1 close needs
        # the stats row.
        t3, sq3 = [], []

        def evac_og(m, ps):
            t = ap_.tile([128, PAD], bf, tag=f"t3{m}", name=f"t3{m}")
            if zbog:
                nc.scalar.activation(t[:], ps[:], AF.Identity)
            else:
                nc.scalar.activation(t[:], ps[:], AF.Identity,
                                     bias=bogc[:, m:m + 1])
            s = ap_.tile([128, PAD], bf, tag=f"sq3{m}", name=f"sq3{m}")
            nc.vector.tensor_mul(s[:], t[:], t[:])
            t3.append(t[:])
            sq3.append(s[:])

        rgrow = rp.tile([1, PAD], bf, tag="rgrow", name="rgrow")
        for g0, gn in ((0, 5), (5, 3)):
            ms = list(range(g0, g0 + gn))
            pss = [pmm.tile([128, PAD], f32, tag="mm", name=f"og{m}")
                   for m in ms]
            for k in range(KH):
                for i, m in enumerate(ms):
                    nc.tensor.matmul(pss[i][:],
                                     wog[k][:, m * 128:(m + 1) * 128],
                                     gated[k], start=(k == 0), stop=False)
            if g0 == 0:
                Sg = pst.tile([64, PAD], f32, tag="st", name="Sg")
                for k in range(4):
                    nc.tensor.matmul(Sg[0:1, :], onesb[:], gated[k],
                                     start=(k == 0), stop=(k == 3))
                for k in range(4, 8):
                    nc.tensor.matmul(Sg[32:33, :], onesb[:], gated[k],
                                     start=(k == 4), stop=(k == 7))
                cg = rp.tile([1, PAD], f32, tag="cg", name="cg")
                nc.vector.tensor_scalar(cg[:], Sg[32:33, :], -1.0 / H,
                                        None, op0=ALU.mult)
                nc.vector.scalar_tensor_tensor(rgrow[:], Sg[0:1, :],
                                               -1.0 / H, cg[:],
                                               op0=ALU.mult, op1=ALU.add)
            for i, m in enumerate(ms):
                nc.tensor.matmul(pss[i][:],
                                 rs[:, H + m * 128:H + (m + 1) * 128],
                                 rgrow[:], start=False, stop=True)
            for i, m in enumerate(ms):
                evac_og(m, pss[i])

        # hoist the sigmoid->abs_rsqrt activation-table swap here (hidden
        # under og/mm2 PE work); without this the auto-inserted load lands
        # after t13, which depends on the late h2 stats
        dumo2 = rp.tile([1, 1], f32, tag="dumo2", name="dumo2")
        nc.scalar.activation(dumo2[:], dum[:], AF.Abs_reciprocal_sqrt,
                             bias=epsE[:])

        # ---------------- mm2 k-MMs interleaved with the h2 stats ---------
        # first half covers the og-evac tail, stats run next, second half
        # covers the A3 row chain; only the rank-1 close needs the rows
        ps2 = [pmm.tile([128, PAD], f32, tag="mm", name=f"y{m}")
               for m in range(KD)]
        for k in range(KH // 2):
            for m in range(KD):
                nc.tensor.matmul(ps2[m][:],
                                 w2[k][:, m * 128:(m + 1) * 128], t3[k],
                                 start=(k == 0), stop=False)

        S3 = stats_pair(t3, sq3, "S3")

        for k in range(KH // 2, KH):
            for m in range(KD):
                nc.tensor.matmul(ps2[m][:],
                                 w2[k][:, m * 128:(m + 1) * 128], t3[k],
                                 start=False, stop=False)
        r2row = rp.tile([1, PAD], bf, tag="r2row", name="r2row")
        nc.vector.tensor_scalar(r2row[:], S3[0:1, :], -1.0 / H, None,
                                op0=ALU.mult)
        t13 = rp.tile([1, PAD], f32, tag="t13", name="t13")
        nc.scalar.activation(t13[:], S3[0:1, :], AF.Square, scale=1.0 / H)
        u3 = rp.tile([1, PAD], f32, tag="u3", name="u3")
        nc.vector.scalar_tensor_tensor(u3[:], S3[32:33, :], 1.0 / H, t13[:],
                                       op0=ALU.mult, op1=ALU.subtract)
        # scalar-engine rsqrt: the act-table swap (sigmoid -> abs_rsqrt,
        # auto-inserted after the last t3 copy) hides under mm_og/mm2
        A3 = rp.tile([1, PAD], bf, tag="A3", name="A3")
        nc.scalar.activation(A3[:], u3[:], AF.Abs_reciprocal_sqrt,
                             bias=epsE[:])
        A3b = ap_.tile([128, PAD], bf, tag="A3b", name="A3b")
        nc.gpsimd.partition_broadcast(A3b[:], A3[:])

        # ---------------- mm2 rank-1 close + output path ------------------
        opre, sq4 = [], []

        def evac_y(m, ps):
            o = ap_.tile([128, PAD], f32r, tag=f"o{m}", name=f"o{m}")
            nc.vector.tensor_mul(o[:], ps[:], A3b[:])
            if not zb2:
                nc.vector.tensor_scalar(o[:], o[:], b2c[:, m:m + 1], None,
                                        op0=ALU.add)
            op = ap_.tile([128, PAD], f32r, tag=f"op{m}", name=f"op{m}")
            nc.vector.scalar_tensor_tensor(op[:], xts[m], 0.1, o[:],
                                           op0=ALU.mult, op1=ALU.add)
            s4 = ap_.tile([128, PAD], f32r, tag=f"s4{m}", name=f"s4{m}")
            nc.scalar.activation(s4[:], op[:], AF.Square)
            opre.append(op[:])
            sq4.append(s4[:])

        for m in range(KD):
            nc.tensor.matmul(ps2[m][:],
                             rs[:, 2 * H + m * 128:2 * H + (m + 1) * 128],
                             r2row[:], start=False, stop=True)
        for m in range(KD):
            evac_y(m, ps2[m])

        # ---------------- final LN (exact, scalar rsqrt) ------------------
        S4a = pst.tile([64, PAD], f32, tag="st", name="S4a")
        S4b = pst.tile([64, PAD], f32, tag="st", name="S4b")
        for k in range(KD):
            nc.tensor.matmul(S4a[0:1, :], onesfr[:], opre[k],
                             start=(k == 0), stop=(k == KD - 1))
            nc.tensor.matmul(S4b[0:1, :], onesfr[:], sq4[k],
                             start=(k == 0), stop=(k == KD - 1))
        # out = rr4 * (D*opre - s1): broadcast s1 and rr4 separately so the
        # s1 broadcast overlaps the u4/rr4 row chain
        s1s = rp.tile([1, PAD], bf, tag="s1s", name="s1s")
        nc.vector.tensor_copy(s1s[:], S4a[0:1, :])
        s1b = pmm.tile([128, PAD], f32, tag="mm", name="s1b")
        nc.tensor.matmul(s1b[:], onesrb[:], s1s[:], start=True, stop=True)
        t14 = rp.tile([1, PAD], f32, tag="t14", name="t14")
        nc.scalar.activation(t14[:], S4a[0:1, :], AF.Square)
        u4 = rp.tile([1, PAD], f32, tag="u4", name="u4")
        nc.vector.scalar_tensor_tensor(u4[:], S4b[0:1, :], float(D), t14[:],
                                       op0=ALU.mult, op1=ALU.subtract)
        rr4 = rp.tile([1, PAD], bf, tag="rr4", name="rr4")
        nc.scalar.activation(rr4[:], u4[:], AF.Abs_reciprocal_sqrt,
                             bias=epsD[:])
        r4b = pmm.tile([128, PAD], f32, tag="mm", name="r4b")
        nc.tensor.matmul(r4b[:], onesrb[:], rr4[:], start=True, stop=True)
        for k in range(KD):
            ot = ap_.tile([128, PAD], bf, tag=f"ot{k}", name=f"ot{k}")
            nc.vector.scalar_tensor_tensor(ot[:], opre[k], float(D),
                                           s1b[:], op0=ALU.mult,
                                           op1=ALU.subtract)
            nc.vector.tensor_mul(ot[:], ot[:], r4b[:])
            (nc.sync if k == 0 else nc.scalar).dma_start(
                out_d.ap()[k * 128:(k + 1) * 128, :], ot[:])

    nc.compile()
    return nc


def _get_nc(PAD, zbg, zbog, zb2):
    key = (PAD, zbg, zbog, zb2)
    if key not in _cache:
        _cache[key] = _build(PAD, zbg, zbog, zb2)
    return _cache[key]


def _bf16(a):
    import ml_dtypes
    return np.ascontiguousarray(
        np.asarray(a, dtype=np.float32).astype(ml_dtypes.bfloat16))


def _f8(a, scale):
    import ml_dtypes
    return np.ascontiguousarray(
        (np.asarray(a, dtype=np.float32) * scale).astype(
            ml_dtypes.float8_e4m3))


def _flatk(w):
    """[K*128, F] -> [128, K*F]: per-partition concat of the K k-tiles so
    the device DMA is one big contiguous descriptor per partition."""
    K = w.shape[0] // 128
    return np.ascontiguousarray(
        w.reshape(K, 128, -1).transpose(1, 0, 2).reshape(128, -1))


def _numpy_ref(x, cat_ids, W0, b0, Wm, bm, Wg, bg, Wog, bog, W2, b2):
    """Host fallback for the (never-hit) nonzero b0/bm case."""
    def ln(v):
        m = v.mean(-1, keepdims=True)
        s = ((v - m) ** 2).mean(-1, keepdims=True)
        return (v - m) / np.sqrt(s + EPS)

    x = np.asarray(x, np.float32)
    cid = np.asarray(cat_ids).astype(np.int64).ravel()
    xn = ln(x)
    out = np.zeros_like(x)
    for c in range(N_CORES):
        idx = np.where(cid == c)[0]
        if len(idx) == 0:
            continue
        h = np.maximum(xn[idx] @ W0[c] + b0[c], 0)
        main = h @ Wm[c] + bm[c]
        gate = h @ Wg[c] + bg[c]
        g = ln(main * (1.0 / (1.0 + np.exp(-gate))))
        h = ln(g @ Wog[c] + bog[c])
        out[idx] = h @ W2[c] + b2[c]
    return ln(out + 0.1 * x).astype(np.float32)


def _prep(x, cat_ids, W0, b0, Wm, bm, Wg, bg, Wog, bog, W2, b2):
    x = np.ascontiguousarray(np.asarray(x, dtype=np.float32))
    cid = np.asarray(cat_ids).astype(np.int64).ravel()
    counts = np.bincount(cid, minlength=N_CORES)
    PAD = int(max(PAD_MIN, ((counts.max() + 31) // 32) * 32))
    order = np.argsort(cid, kind="stable")
    starts = np.zeros(N_CORES + 1, np.int64)
    starts[1:] = np.cumsum(counts)

    zbg = not np.any(np.asarray(bg))
    zbog = not np.any(np.asarray(bog))
    zb2 = not np.any(np.asarray(b2))
    need_bc = (not zbg) or (not zbog) or (not zb2)

    in_maps = []
    for c in range(N_CORES):
        ids = order[starts[c]:starts[c + 1]]
        xc = np.zeros((PAD, D), np.float32)
        xc[:len(ids)] = x[ids]
        w0b = _bf16(W0[c])
        wogb = _bf16(Wog[c])
        w2b = _bf16(W2[c])
        rsum = np.concatenate([
            w0b.astype(np.float32).sum(0),
            wogb.astype(np.float32).sum(0),
            w2b.astype(np.float32).sum(0),
        ])[None, :]
        m = {
            "XT": _flatk(_bf16(xc.T)),
            "W0": _flatk(w0b),
            "Wm": _flatk(_bf16(Wm[c])),
            "Wg": _flatk(_f8(Wg[c], 8.0)),
            "Wog": _flatk(wogb), "W2": _flatk(w2b),
            "RS": _bf16(rsum),
        }
        if need_bc:
            bc = np.concatenate([
                np.asarray(bg[c], np.float32).reshape(KH, 128).T,
                np.asarray(bog[c], np.float32).reshape(KH, 128).T,
                np.asarray(b2[c], np.float32).reshape(KD, 128).T,
            ], axis=1)
            m["BC"] = np.ascontiguousarray(bc)
        in_maps.append(m)
    return in_maps, order, starts, PAD, (zbg, zbog, zb2), x.shape[0]


def kernel(x, cat_ids, W0, b0, Wm, bm, Wg, bg, Wog, bog, W2, b2,
           **run_kwargs):
    if np.any(np.asarray(b0)) or np.any(np.asarray(bm)):
        return _numpy_ref(x, cat_ids, W0, b0, Wm, bm, Wg, bg, Wog, bog,
                          W2, b2)
    from concourse.bass_utils import run_bass_kernel_spmd

    in_maps, order, starts, PAD, flags, N = _prep(
        x, cat_ids, W0, b0, Wm, bm, Wg, bg, Wog, bog, W2, b2)
    nc = _get_nc(PAD, *flags)
    res = run_bass_kernel_spmd(nc, in_maps, core_ids=list(range(N_CORES)),
                               **run_kwargs)
    out = np.zeros((N, D), np.float32)
    for c in range(N_CORES):
        ids = order[starts[c]:starts[c + 1]]
        out[ids] = res.results[c]["out"].astype(np.float32).T[:len(ids)]
    if run_kwargs:
        kernel.last_results = res
    return out
